# revision 1
# baseline (speedup 1.0000x reference)
"""Trainium2 Bass kernel for nn_DCondJastrow (B=16384, N=16, DIM=2).

Data-parallel over 8 NeuronCores: batch is split into 8 shards of 2048
walkers; all MLP weights are replicated (tiny).

Per-core dataflow (Bc = 2048 walkers):
  - inputs land as feature-major tiles: xx/yy [16, Bc], de [16, Bc]
  - pair differences dx/dy [120, Bc] via one +-1 selection matmul
  - per-pair scalar features (rij, log1p, 4 exps, r/(1+r)) computed with
    pairs on the partition axis (full-lane ACT/DVE work)
  - features are relayouted through a DRAM staging buffer into rhs
    tiles [12, pairs*Bc] (feature on partition, row on free) so the
    6->64->64 MLP runs as full-width matmuls; the two stacked groups of
    64 hidden units process two pair-blocks per pass
  - hidden GELUs on the ACT engine (exact-erf Gelu table), bf16 matmuls
  - the pair/node mean plus the rho readout layer are folded into one
    accumulated matmul (L3 is linear, so mean-then-L3 == L3-then-mean,
    and L3 composes with rho_W0 on the host)
  - the cusp sum over pairs rides ones-vector matmuls (bf16 hi/lo split
    for fp32-grade accuracy) into the same final PSUM group as rho_W1
"""

import numpy as np
import ml_dtypes

B, N, DIM = 16384, 16, 2
H, DL, DEMB = 64, 5, 16
NCORES = 8
BC = B // NCORES          # 2048 walkers per core
P = N * (N - 1) // 2      # 120 pairs
PG = P // 2               # 60 pairs per stacked group
NG = N // 2               # 8 nodes per stacked group
SLABS = [2, 4, 6, 12, 12, 12, 12]   # pairs per reload slab (per group)
BF16 = ml_dtypes.bfloat16

_CACHE = {}

# bf16 blob column layout
_W1PSI, _W1PHI, _W0PSI, _W0PHI, _WR1, _ONES, _WFPHIB, _WFPSIB, _WDEB = (
    0, 128, 256, 384, 512, 513, 514, 578, 642)
_WB16_COLS = 706
# f32 blob column layout
_WFPHI, _WFPSI, _B0PSI, _B1PSI, _B0PHI, _B1PHI, _BRHO, _EPSP, _EPSN, _DSEL = (
    0, 64, 128, 129, 130, 131, 132, 133, 134, 135)
_WDE32 = 255
_WF32_COLS = 319


def _build_program(weights):
    import concourse.mybir as mybir
    from concourse import bacc
    from concourse.tile import TileContext

    dt = mybir.dt
    AF = mybir.ActivationFunctionType
    ALU = mybir.AluOpType

    nc = bacc.Bacc("TRN2", target_bir_lowering=False, debug=False)

    def din(name, shape, dtype=dt.float32):
        return nc.dram_tensor(name, list(shape), dtype, kind="ExternalInput").ap()

    xx_d = din("xx", (N, BC))
    yy_d = din("yy", (N, BC))
    de_d = din("de", (DEMB, BC))
    wb16_d = din("wb16", (128, _WB16_COLS), dt.bfloat16)
    wf32_d = din("wf32", (128, _WF32_COLS))
    out_d = nc.dram_tensor("out", [1, BC], dt.float32, kind="ExternalOutput").ap()

    rho_b1 = float(weights["rho_b1"][0])

    with TileContext(nc) as tc:
        with (
            tc.tile_pool(name="const", bufs=1) as cpool,
            tc.tile_pool(name="persist", bufs=1) as ppool,
            tc.tile_pool(name="acc", bufs=2) as apool,
            tc.tile_pool(name="psum", bufs=1, space="PSUM") as pspool,
        ):
            wb16 = cpool.tile([128, _WB16_COLS], dt.bfloat16, tag="wb16")
            nc.sync.dma_start(wb16[:], wb16_d)
            wf32 = cpool.tile([128, _WF32_COLS], dt.float32, tag="wf32")
            nc.sync.dma_start(wf32[:], wf32_d)

            w1psi = wb16[:, _W1PSI : _W1PSI + 128]
            w1phi = wb16[:, _W1PHI : _W1PHI + 128]
            w0psi = wb16[0:12, _W0PSI : _W0PSI + 128]
            w0phi = wb16[0:6, _W0PHI : _W0PHI + 128]
            wr1 = wb16[0:H, _WR1 : _WR1 + 1]
            onesb = wb16[0:P, _ONES : _ONES + 1]
            wfphi = wb16[:, _WFPHIB : _WFPHIB + H]
            wfpsi = wb16[:, _WFPSIB : _WFPSIB + H]
            wdeb = wb16[0:DEMB, _WDEB : _WDEB + H]

            def bias(col, rows=128):
                return wf32[0:rows, col : col + 1]

            def dsel4(i):
                return wf32[32 * i : 32 * i + N, _DSEL : _DSEL + P]

            de = ppool.tile([DEMB, BC], dt.float32, tag="de")
            nc.sync.dma_start(de[:], de_d)
            deb = ppool.tile([DEMB, BC], dt.bfloat16, tag="deb")
            nc.vector.tensor_copy(deb[:], de[:])
            # cusp integrand, bf16 hi/lo split (hi + lo == ce to ~2^-18 rel)
            ce_hi = ppool.tile([P, BC], dt.bfloat16, tag="ce_hi")
            ce_lo = ppool.tile([P, BC], dt.bfloat16, tag="ce_lo")

            # DRAM staging for the feature->row-major relayout: one bulk DMA
            # per stacked group out, one DMA per slab back in (keeps the
            # per-instruction sync-wait count tiny).
            frhs_d = nc.dram_tensor("frhs", [12, PG * BC], dt.bfloat16).ap()
            fphi_d = nc.dram_tensor("fphi", [6, NG * BC], dt.bfloat16).ap()

            # ---------------- feature phase ----------------
            with (
                tc.tile_pool(name="feat", bufs=1) as fpool,
                tc.tile_pool(name="scr", bufs=6) as spool,
                tc.tile_pool(name="scrs", bufs=3) as sspool,
            ):
                x4 = fpool.tile([96 + N, BC], dt.float32, tag="x4")
                y4 = fpool.tile([96 + N, BC], dt.float32, tag="y4")
                for i in range(4):
                    nc.sync.dma_start(x4[32 * i : 32 * i + N, :], xx_d)
                    nc.sync.dma_start(y4[32 * i : 32 * i + N, :], yy_d)
                xx = x4[0:N, :]
                yy = y4[0:N, :]
                # feature k of pair p lives at fbig[p, k*BC:(k+1)*BC]
                fbig = fpool.tile([P, 6 * BC], dt.bfloat16, tag="fbig")
                # node features: x / y / r^2 side by side
                xyrb = fpool.tile([N, 3 * BC], dt.bfloat16, tag="xyrb")

                def scr():
                    return spool.tile([P, BC], dt.float32, tag="scratch", name="scratch")

                # pair differences via +-1 selection matmul (fp32)
                psdx = pspool.tile([P, BC], dt.float32, tag="ps1", name="psdx")
                psdy = pspool.tile([P, BC], dt.float32, tag="ps2", name="psdy")
                for i in range(4):
                    s = slice(i * 512, (i + 1) * 512)
                    nc.tensor.matmul(
                        psdx[:, s], dsel4(i), x4[32 * i : 32 * i + N, s],
                        tile_position=(32 * i, 0),
                    )
                for i in range(4):
                    s = slice(i * 512, (i + 1) * 512)
                    nc.tensor.matmul(
                        psdy[:, s], dsel4(i), y4[32 * i : 32 * i + N, s],
                        tile_position=(32 * i, 0),
                    )

                t1 = scr()
                nc.scalar.activation(t1[:, 0:1024], psdx[:, 0:1024], AF.Square)
                nc.scalar.activation(t1[:, 1024:], psdx[:, 1024:], AF.Square)
                t2 = scr()
                nc.scalar.activation(t2[:, 0:1024], psdy[:, 0:1024], AF.Square)
                nc.scalar.activation(t2[:, 1024:], psdy[:, 1024:], AF.Square)
                r2p = scr()
                nc.vector.tensor_add(r2p[:], t1[:], t2[:])

                # node r^2 (phi feature) + bf16 casts of node coords
                sqx = sspool.tile([N, BC], dt.float32, tag="sscr")
                nc.vector.tensor_mul(sqx[:], xx[:], xx[:])
                sqy = sspool.tile([N, BC], dt.float32, tag="sscr")
                nc.vector.tensor_mul(sqy[:], yy[:], yy[:])
                r2n = sspool.tile([N, BC], dt.float32, tag="sscr")
                nc.vector.tensor_add(r2n[:], sqx[:], sqy[:])
                nc.vector.tensor_copy(xyrb[:, 0:BC], xx[:])
                nc.vector.tensor_copy(xyrb[:, BC : 2 * BC], yy[:])
                nc.vector.tensor_copy(xyrb[:, 2 * BC :], r2n[:])
                nc.sync.dma_start(
                    fphi_d[0:3, :].rearrange("k (n b) -> n k b", n=NG),
                    xyrb[0:NG, :].rearrange("n (k b) -> n k b", k=3),
                )
                nc.sync.dma_start(
                    fphi_d[3:6, :].rearrange("k (n b) -> n k b", n=NG),
                    xyrb[NG:N, :].rearrange("n (k b) -> n k b", k=3),
                )

                # rij = sqrt(r2p + 1e-12), one Newton step for table error
                rij0 = scr()
                nc.scalar.activation(rij0[:], r2p[:], AF.Sqrt, bias=bias(_EPSP, P))

                def fcol(k):
                    return fbig[:, k * BC : (k + 1) * BC]

                # exp(-r^2) only needs r2p: issue it while the Newton step runs
                nc.scalar.activation(fcol(2), r2p[:], AF.Exp, scale=-1.0, bias=bias(_EPSN, P))
                rec = scr()
                nc.vector.reciprocal(rec[:], rij0[:])
                tq = scr()
                nc.vector.scalar_tensor_tensor(
                    tq[:], r2p[:], 1e-12, rec[:], op0=ALU.add, op1=ALU.mult
                )
                rijh = scr()
                nc.vector.tensor_add(rijh[:], rij0[:], tq[:])
                rij = scr()
                nc.vector.tensor_scalar_mul(rij[:], rijh[:], 0.5)

                # remaining psi features (bf16 out); Ln last so the exp and
                # ln table sets each load exactly once
                nc.scalar.activation(fcol(3), rij[:], AF.Exp, scale=-0.5)
                nc.scalar.activation(fcol(4), rij[:], AF.Exp, scale=-1.0)
                nc.scalar.activation(fcol(5), rij[:], AF.Exp, scale=-2.0)
                u1 = scr()
                nc.vector.tensor_scalar_add(u1[:], rij[:], 1.0)
                v1 = scr()
                nc.vector.reciprocal(v1[:], u1[:])
                nc.vector.tensor_mul(fcol(1), rij[:], v1[:])
                f4f = scr()
                nc.scalar.activation(f4f[:], rij[:], AF.Exp, scale=-1.0)
                nc.scalar.activation(fcol(0), rij[:], AF.Ln, bias=1.0)
                ce = scr()
                nc.vector.tensor_mul(ce[:], rij[:], f4f[:])
                nc.vector.tensor_copy(ce_hi[:], ce[:])
                ce_r = scr()
                nc.vector.tensor_sub(ce_r[:], ce[:], ce_hi[:])
                nc.vector.tensor_copy(ce_lo[:], ce_r[:])

                # stage the relayouted features to DRAM (row r=k+6g holds
                # feature k of group g, columns ordered pair-major)
                for k in (2, 3, 4, 5, 0, 1):
                    for g in range(2):
                        nc.gpsimd.dma_start(
                            frhs_d[k + 6 * g : k + 6 * g + 1, :].rearrange(
                                "o (p b) -> (o p) b", b=BC
                            ),
                            fbig[g * PG : (g + 1) * PG,
                                 k * BC : (k + 1) * BC],
                        )

            # ---------------- MLP phases ----------------
            ps1 = pspool.tile([128, BC], dt.float32, tag="ps1", name="ps1")
            ps2 = pspool.tile([128, BC], dt.float32, tag="ps2", name="ps2")

            with (
                tc.tile_pool(name="rhs", bufs=2) as rpool,
                tc.tile_pool(name="rhsp", bufs=1) as rppool,
                tc.tile_pool(name="hid", bufs=3) as hpool,
            ):
                acc_psi = apool.tile([128, BC], dt.float32, tag="accpsi")
                nc.vector.memset(acc_psi[:], 0.0)
                acc_phi = apool.tile([128, BC], dt.float32, tag="accphi")
                nc.vector.memset(acc_phi[:], 0.0)
                accs = {"phi": acc_phi, "psi": acc_psi}
                remaining = {"phi": NG, "psi": PG}

                def gen_chunks():
                    # phi node MLP first: covers the psi staging/reload latency
                    rhsp = rppool.tile([6, NG * BC], dt.bfloat16, tag="rhsphi")
                    for j in range(4):
                        cs = slice(j * 2 * BC, (j + 1) * 2 * BC)
                        nc.gpsimd.dma_start(rhsp[:, cs], fphi_d[:, cs])
                    for c in range(NG):
                        yield (rhsp[:, c * BC : (c + 1) * BC],
                               w0phi, w1phi, _B0PHI, _B1PHI, "phi")
                    off = 0
                    for slab in SLABS:
                        rhs1 = rpool.tile(
                            [12, max(SLABS) * BC], dt.bfloat16, tag="rhs1", name="rhs1"
                        )
                        nc.sync.dma_start(
                            rhs1[:, : slab * BC],
                            frhs_d[:, off * BC : (off + slab) * BC],
                        )
                        for c in range(slab):
                            yield (rhs1[:, c * BC : (c + 1) * BC],
                                   w0psi, w1psi, _B0PSI, _B1PSI, "psi")
                        off += slab

                def front(rhs_c, w0, b0c):
                    for q in range(BC // 512):
                        s = slice(q * 512, (q + 1) * 512)
                        nc.tensor.matmul(ps1[:, s], w0, rhs_c[:, s])
                    h1 = hpool.tile([128, BC], dt.bfloat16, tag="h1", name="h1")
                    nc.scalar.activation(h1[:], ps1[:], AF.Gelu, bias=bias(b0c))
                    return h1

                def back(h1, w1, b1c, stream):
                    for q in range(BC // 512):
                        s = slice(q * 512, (q + 1) * 512)
                        nc.tensor.matmul(ps2[:, s], w1, h1[:, s])
                    h2 = hpool.tile([128, BC], dt.bfloat16, tag="h2", name="h2")
                    nc.scalar.activation(h2[:], ps2[:], AF.Gelu, bias=bias(b1c))
                    remaining[stream] -= 1
                    odt = dt.float32 if remaining[stream] else dt.bfloat16
                    nxt = apool.tile([128, BC], odt,
                                     tag=f"acc{stream}", name=f"acc{stream}")
                    nc.vector.tensor_add(nxt[:], accs[stream][:], h2[:])
                    accs[stream] = nxt

                # software pipeline: GELU1 of chunk c+1 issues ahead of GELU2
                # of chunk c so the ACT engine never waits on the PE
                prev = None
                for rhs_c, w0, w1, b0c, b1c, stream in gen_chunks():
                    h1 = front(rhs_c, w0, b0c)
                    if prev is not None:
                        back(*prev)
                    prev = (h1, w1, b1c, stream)
                back(*prev)
                acc_phi = accs["phi"]
                acc_psi = accs["psi"]

            # ---------------- fused readout ----------------
            # pre-act = Wfold_phi^T acc_phi + Wfold_psi^T acc_psi + wde^T de
            # (the stacked-half fold, the /16 and /120 means, the phi/psi L3
            #  layers and their biases are all baked into the host weights)
            with tc.tile_pool(name="ro", bufs=1) as ropool:
                for q in range(BC // 512):
                    s = slice(q * 512, (q + 1) * 512)
                    nc.tensor.matmul(
                        ps1[0:H, s], wfphi, acc_phi[:, s], start=True, stop=False
                    )
                    nc.tensor.matmul(
                        ps1[0:H, s], wfpsi, acc_psi[:, s], start=False, stop=False
                    )
                    nc.tensor.matmul(
                        ps1[0:H, s], wdeb, deb[:, s], start=False, stop=True
                    )
                hr = ropool.tile([H, BC], dt.bfloat16, tag="hr")
                nc.scalar.activation(hr[:], ps1[0:H, :], AF.Gelu, bias=bias(_BRHO, H))
                for q in range(BC // 512):
                    s = slice(q * 512, (q + 1) * 512)
                    nc.tensor.matmul(ps2[0:1, s], wr1, hr[:, s], start=True, stop=False)
                    nc.tensor.matmul(ps2[0:1, s], onesb, ce_hi[:, s], start=False, stop=False)
                    nc.tensor.matmul(ps2[0:1, s], onesb, ce_lo[:, s], start=False, stop=True)
                outsb = ropool.tile([1, BC], dt.float32, tag="outsb")
                nc.scalar.activation(outsb[:], ps2[0:1, :], AF.Copy, bias=rho_b1)
                nc.sync.dma_start(out_d, outsb[:])

    if not nc.is_finalized():
        nc.finalize()
    return nc


def _prep_weights(inputs):
    f32 = np.float32
    w = {k: np.asarray(v, dtype=f32) for k, v in inputs.items() if k not in ("x", "d_emb")}

    iu, ju = np.triu_indices(N, 1)
    dsel = np.zeros((N, P), f32)
    dsel[iu, np.arange(P)] = 1.0
    dsel[ju, np.arange(P)] = -1.0

    def stackw(w0):  # [k, H] -> [2k, 128] block-diagonal over the two groups
        k = w0.shape[0]
        out = np.zeros((2 * k, 128), f32)
        out[0:k, 0:H] = w0
        out[k:, H:] = w0
        return out

    rho_W0 = w["rho_W0"]
    wfphi = np.vstack([w["phi_W2"], w["phi_W2"]]) / N @ rho_W0[0:DL]
    wfpsi = np.vstack([w["psi_W2"], w["psi_W2"]]) / P @ rho_W0[DL : 2 * DL]
    brho = (
        w["rho_b0"]
        + w["phi_b2"] @ rho_W0[0:DL]
        + w["psi_b2"] @ rho_W0[DL : 2 * DL]
    )

    wb16 = np.zeros((128, _WB16_COLS), f32)
    wb16[0:128, _W1PSI : _W1PSI + 128] = stackw(w["psi_W1"])[0:128]
    wb16[0:128, _W1PHI : _W1PHI + 128] = stackw(w["phi_W1"])[0:128]
    wb16[0:12, _W0PSI : _W0PSI + 128] = stackw(w["psi_W0"])
    wb16[0:6, _W0PHI : _W0PHI + 128] = stackw(w["phi_W0"])
    wb16[0:H, _WR1 : _WR1 + 1] = w["rho_W1"]
    wb16[0:P, _ONES : _ONES + 1] = 1.0
    wb16[0:128, _WFPHIB : _WFPHIB + H] = wfphi.reshape(128, H)
    wb16[0:128, _WFPSIB : _WFPSIB + H] = wfpsi.reshape(128, H)
    wb16[0:DEMB, _WDEB : _WDEB + H] = rho_W0[2 * DL :]

    wf32 = np.zeros((128, _WF32_COLS), f32)
    wf32[0:128, _WFPHI : _WFPHI + H] = wfphi.reshape(128, H)
    wf32[0:128, _WFPSI : _WFPSI + H] = wfpsi.reshape(128, H)
    wf32[0:128, _B0PSI] = np.tile(w["psi_b0"], 2)
    wf32[0:128, _B1PSI] = np.tile(w["psi_b1"], 2)
    wf32[0:128, _B0PHI] = np.tile(w["phi_b0"], 2)
    wf32[0:128, _B1PHI] = np.tile(w["phi_b1"], 2)
    wf32[0:H, _BRHO] = brho
    wf32[:, _EPSP] = 1e-12
    wf32[:, _EPSN] = -1e-12
    for i in range(4):
        wf32[32 * i : 32 * i + N, _DSEL : _DSEL + P] = dsel

    return {
        "wb16": wb16.astype(BF16),
        "wf32": wf32,
        "rho_b1": w["rho_b1"],
    }


def kernel(**inputs):
    from concourse.bass_utils import run_bass_kernel_spmd

    x = np.ascontiguousarray(np.asarray(inputs["x"], dtype=np.float32))
    d_emb = np.ascontiguousarray(np.asarray(inputs["d_emb"], dtype=np.float32))
    assert x.shape == (B, N, DIM) and d_emb.shape == (B, DEMB)

    wmap = _prep_weights(inputs)
    rho_b1_key = float(wmap["rho_b1"][0])
    if _CACHE.get("rho_b1") != rho_b1_key:
        _CACHE["nc"] = _build_program(wmap)
        _CACHE["rho_b1"] = rho_b1_key
    nc = _CACHE["nc"]

    in_maps = []
    for c in range(NCORES):
        xc = x[c * BC : (c + 1) * BC]            # [BC, N, DIM]
        m = {
            "wb16": wmap["wb16"],
            "wf32": wmap["wf32"],
            "xx": np.ascontiguousarray(xc[:, :, 0].T),
            "yy": np.ascontiguousarray(xc[:, :, 1].T),
            "de": np.ascontiguousarray(d_emb[c * BC : (c + 1) * BC].T),
        }
        in_maps.append(m)

    res = run_bass_kernel_spmd(nc, in_maps, list(range(NCORES)))
    out = np.concatenate([r["out"].reshape(BC) for r in res.results])
    return out.astype(np.float32)

